# revision 37
# baseline (speedup 1.0000x reference)
"""Distributed Bass kernel for nn_AttentionCircuit (B=2,S=2048,D=2048,RANK=512,H=16).

Sharding: 8 cores = 2 batches x 4 group-positions. Core (b, g) computes
attention for head-group g (4 heads / 512 D-cols) of batch b over all S.

Two collectives per chunk, both off the critical path:
  - The gated low-rank projection t (A-stage) is rank-sharded: each core
    computes its own 128 rank rows (1/4 of the work) and an AllGather
    rebuilds the full [rank, SC] gated t (chunks 2/3; their AG triggers
    are emitted in iter 0 AFTER the first A2A so they queue behind it on
    the trigger-ordered CC stream). Chunks 0/1 are instead computed
    REPLICATED (full rank locally, read matrices staged in the
    not-yet-loaded V_sb/wo_sb regions) so B(0)/B(1)/C(0) never wait on
    the ~30-70us first-collective NRT barrier.
  - Instead of a ReduceScatter of [SC, D] W_O partials (the last one an
    unoverlapped ~64us tail), each chunk's attention output ao is
    AllToAll'd (over all 8 cores -- 4-core A2A is unsupported -- with
    each peer-slice dual-written to both candidate rank slots and the
    receiver keeping its own group's half via a per-core 0/1 gsel input)
    and every core computes its own 128 output rows against a full SBUF
    copy of W_O. Each A2A is split in two head-pair halves (the first
    fires mid-C(t) and is fully hidden), D(t) is deferred one chunk so
    the exchange is long done when it runs, and B(t+1) fills the A2A(t)
    window. Both collective paths (A2A at real payload size, AG) are
    warmed during the barrier shadow -- the first large A2A otherwise
    pays a ~30us ramp.
All matmul operands bf16 (1 cyc/row on PE, half the DMA bytes of fp32).
Bulk loads (x, gates, weights, W_O) go through the Activation HWDGE queue
so latency-critical collective DMAs on the SP queue never sit behind them.

Per-core, per 512-wide s-chunk: A (own rank rows of gated t^T) -> AG;
B: Q^T/K^T[own cols]/V from gathered t; C: scores^T = K^T.T Q^T -> exp ->
causal mask (block skip + static masks on diagonal) -> rowsum via
ones-matmul of DVE quad-sums (lagged one pair) -> PV -> normalize via
fast fp32 1/Z + bcast matmul (1/0.81 folded into W_O on host);
A2A: ao -> aoT_full [dh, 16 global heads, 128 own queries];
D: own out rows = aoT_full.T @ W_O (full 2048 contraction).
Host reassembles: core (b, g) holds rows t*512+g*128..+128 of batch b.
"""
import sys
import numpy as np
import ml_dtypes

sys.path.insert(0, '/opt/trn_rl_repo')

import concourse.bass as bass  # noqa: E402
from concourse import bacc  # noqa: E402
import concourse.mybir as mybir  # noqa: E402
import concourse.tile as tile  # noqa: E402
from concourse.bass_utils import run_bass_kernel_spmd  # noqa: E402

B, S, D = 2, 2048, 2048
RANK = 512
NH = 16
HG = 4              # heads per core / group size
DHG = D // HG       # 512 cols per core
P = 128
DB = D // P         # 16 d-blocks
RB = RANK // P      # 4 rank-blocks (== own-col blocks)
SC = S // 4         # 512: s-chunk width == t-chunk width
NT = S // SC        # 4 chunks

F32 = mybir.dt.float32
BF = mybir.dt.bfloat16
AF = mybir.ActivationFunctionType
ALU = mybir.AluOpType

EXP_SCALE = 1.0 / float(np.sqrt(P))
INV_KEEP2 = 1.0 / (0.9 * 0.9)
RGROUPS = [[0, 1, 2, 3], [4, 5, 6, 7]]

_CACHE = {}


def _r(ap):
    """[ (o p), f ] DRAM tensor -> [p, o, f] partition-tiled view."""
    return ap.rearrange("(o p) f -> p o f", p=P)


def _build():
    nc = bacc.Bacc("TRN2", target_bir_lowering=False, debug=False,
                   enable_asserts=False, num_devices=8)
    dt_ = nc.dram_tensor
    xT = dt_("xT", [D, S], BF, kind="ExternalInput").ap()
    gqT = dt_("gqT", [P, S], BF, kind="ExternalInput").ap()
    gkT = dt_("gkT", [P, S], BF, kind="ExternalInput").ap()
    gvT = dt_("gvT", [P, S], BF, kind="ExternalInput").ap()
    qk_readT = dt_("qk_readT", [D, P], BF, kind="ExternalInput").ap()
    v_readT = dt_("v_readT", [D, P], BF, kind="ExternalInput").ap()
    qk_readTf = dt_("qk_readTf", [D, RANK], BF, kind="ExternalInput").ap()
    v_readTf = dt_("v_readTf", [D, RANK], BF, kind="ExternalInput").ap()
    gq0 = dt_("gq0", [RANK, 2 * SC], BF, kind="ExternalInput").ap()
    gk0 = dt_("gk0", [RANK, 2 * SC], BF, kind="ExternalInput").ap()
    gv0 = dt_("gv0", [RANK, 2 * SC], BF, kind="ExternalInput").ap()
    qk_w = dt_("qk_w", [RANK, DHG], BF, kind="ExternalInput").ap()
    v_w = dt_("v_w", [RANK, DHG], BF, kind="ExternalInput").ap()
    wo_full = dt_("wo_full", [D, D], BF, kind="ExternalInput").ap()
    gsel = dt_("gsel", [P, 2], F32, kind="ExternalInput").ap()
    out = dt_("out", [NT, P, D], BF, kind="ExternalOutput").ap()

    with tile.TileContext(nc) as tc:
        _body(tc, dict(
            xT=xT, gqT=gqT, gkT=gkT, gvT=gvT, qk_readT=qk_readT,
            v_readT=v_readT, qk_readTf=qk_readTf, v_readTf=v_readTf,
            gq0=gq0, gk0=gk0, gv0=gv0, qk_w=qk_w, v_w=v_w,
            wo_full=wo_full, gsel=gsel, out=out))
    nc.compile()
    return nc


def _body(tc, io):
    nc = tc.nc
    import contextlib
    ctx = contextlib.ExitStack()
    with ctx:
        pool_main = ctx.enter_context(tc.tile_pool(name="main", bufs=1))
        pool_x = ctx.enter_context(tc.tile_pool(name="x", bufs=1))
        pool_g = ctx.enter_context(tc.tile_pool(name="g", bufs=1))
        pool_agin = ctx.enter_context(tc.tile_pool(name="agin", bufs=1))
        pool_tg = ctx.enter_context(tc.tile_pool(name="tg", bufs=2))
        pool_qt = ctx.enter_context(tc.tile_pool(name="qt", bufs=1))
        pool_ao = ctx.enter_context(tc.tile_pool(name="ao", bufs=1))
        pool_aot = ctx.enter_context(tc.tile_pool(name="aot", bufs=2))
        pool_ao2 = ctx.enter_context(tc.tile_pool(name="ao2", bufs=1))
        pool_osb = ctx.enter_context(tc.tile_pool(name="osb", bufs=1))
        pool_et = ctx.enter_context(tc.tile_pool(name="et", bufs=6))
        pool_ets = ctx.enter_context(tc.tile_pool(name="ets", bufs=2))
        pool_sm = ctx.enter_context(tc.tile_pool(name="sm", bufs=1))
        pool_dram = ctx.enter_context(tc.tile_pool(name="dramb", bufs=1,
                                                   space="DRAM"))
        psSC = ctx.enter_context(tc.tile_pool(name="psSC", bufs=5, space="PSUM"))
        psPV = ctx.enter_context(tc.tile_pool(name="psPV", bufs=2, space="PSUM"))
        psRS = ctx.enter_context(tc.tile_pool(name="psRS", bufs=1, space="PSUM"))

        # warm up the collective stream first: absorbs the ~45us NRT
        # first-collective barrier while the prologue computes chunk 0
        warm_in = pool_dram.tile([P, 1024], BF)
        warm_out = pool_dram.tile([RB, P, 1024], BF)
        warm2_in = pool_dram.tile([8, P, 2, P], BF)
        warm2_out = pool_dram.tile([8, P, 2, P], BF)
        nc.gpsimd.collective_compute(
            "AllToAll", ALU.bypass, ins=[warm2_in.opt()],
            outs=[warm2_out.opt()], replica_groups=[list(range(8))])
        nc.gpsimd.collective_compute(
            "AllGather", ALU.bypass, ins=[warm_in.opt()],
            outs=[warm_out.opt()], replica_groups=RGROUPS)

        # ---- long-lived tensors / constants
        KT_sb = pool_main.tile([P, HG, NT, SC], BF)   # K^T [dh, head, chunk, s]
        V_sb = pool_main.tile([P, DB, DHG], BF)       # V [s-block, own cols]
        wo_sb = pool_main.tile([P, DB, D], BF)        # full W_O (row-tiled)
        qr = pool_main.tile([P, DB, P], BF)           # own 128 rank cols
        vr = pool_main.tile([P, DB, P], BF)
        qw = pool_main.tile([P, RB, DHG], BF)
        vw = pool_main.tile([P, RB, DHG], BF)
        masks = pool_main.tile([P, SC + 3 * P], BF)   # sliding causal mask
        ones_r = pool_main.tile([P, 1], BF)
        onecol = pool_main.tile([1, P], BF)
        gsel_sb = pool_main.tile([P, 2], F32)

        nc.vector.memset(masks[:], 1.0)
        nc.gpsimd.affine_select(
            out=masks[:], in_=masks[:],
            compare_op=ALU.is_ge, fill=0.0, base=-3 * P,
            pattern=[[1, SC + 3 * P]], channel_multiplier=-1)
        nc.vector.memset(ones_r[:], 1.0)
        nc.vector.memset(onecol[:], 1.0)

        ag_in_d = pool_dram.tile([NT, P, 3, SC], BF)
        ag_out_d = pool_dram.tile([NT, RB, P, 3, SC], BF)
        a2a_in0 = pool_dram.tile([NT, 8, P, 2, P], BF, tag="a2a_in0")
        a2a_in1 = pool_dram.tile([NT, 8, P, 2, P], BF, tag="a2a_in1")
        a2a_out0 = pool_dram.tile([NT, 8, P, 2, P], BF, tag="a2a_out0")
        a2a_out1 = pool_dram.tile([NT, 8, P, 2, P], BF, tag="a2a_out1")
        a2a_in = [a2a_in0, a2a_in1]
        a2a_out = [a2a_out0, a2a_out1]

        def dma_chunk_inputs(t, gates=True):
            csl = slice(t * SC, (t + 1) * SC)
            xt = pool_x.tile([P, DB, SC], BF, tag="xt")
            nc.scalar.dma_start(xt[:, :DB // 2, :], _r(io['xT'])[:, :DB // 2, csl])
            nc.scalar.dma_start(xt[:, DB // 2:, :], _r(io['xT'])[:, DB // 2:, csl])
            if not gates:
                return xt, None, None, None
            gq = pool_g.tile([P, SC], BF, tag="gq")
            nc.scalar.dma_start(gq[:], io['gqT'][:, csl])
            gk = pool_g.tile([P, SC], BF, tag="gk")
            nc.scalar.dma_start(gk[:], io['gkT'][:, csl])
            gv = pool_g.tile([P, SC], BF, tag="gv")
            nc.scalar.dma_start(gv[:], io['gvT'][:, csl])
            return xt, gq, gk, gv

        def stage_a(t, ins):
            """Own 128 rank rows of gated t^T for s-chunk t -> AllGather."""
            xt, gq, gk, gv = ins
            agi = pool_agin.tile([P, 3, SC], BF, tag="agi")
            ps = psSC.tile([P, SC], F32, tag="sc")
            for db in range(DB):
                nc.tensor.matmul(ps[:], qr[:, db, :], xt[:, db, :],
                                 start=(db == 0), stop=(db == DB - 1))
            nc.vector.tensor_tensor(agi[:, 0, :], ps[:], gq[:], ALU.mult)
            nc.vector.tensor_tensor(agi[:, 1, :], ps[:], gk[:], ALU.mult)
            psv = psSC.tile([P, SC], F32, tag="sc")
            for db in range(DB):
                nc.tensor.matmul(psv[:], vr[:, db, :], xt[:, db, :],
                                 start=(db == 0), stop=(db == DB - 1))
            nc.vector.tensor_tensor(agi[:, 2, :], psv[:], gv[:], ALU.mult)
            nc.sync.dma_start(ag_in_d[t], agi[:])
            nc.gpsimd.collective_compute(
                "AllGather", ALU.bypass, ins=[ag_in_d[t].opt()],
                outs=[ag_out_d[t].opt()], replica_groups=RGROUPS)

        def tg_fetch(t):
            tg = pool_tg.tile([P, RB, 3, SC], BF, tag="tg")
            nc.sync.dma_start(
                tg[:], ag_out_d[t].rearrange("g p c f -> p g c f"))
            return tg

        def stage_b(t, tg, QT_sb):
            """Q^T/K^T [own cols, chunk t], V [chunk t, own cols]."""
            for db in range(RB):
                dsl = slice(db * P, (db + 1) * P)
                psq = psSC.tile([P, SC], F32, tag="sc")
                for rb in range(RB):
                    nc.tensor.matmul(psq[:], qw[:, rb, dsl], tg[:, rb, 0, :],
                                     start=(rb == 0), stop=(rb == RB - 1))
                nc.scalar.activation(QT_sb[:, db, :], psq[:], AF.Copy)
                psk = psSC.tile([P, SC], F32, tag="sc")
                for rb in range(RB):
                    nc.tensor.matmul(psk[:], qw[:, rb, dsl], tg[:, rb, 1, :],
                                     start=(rb == 0), stop=(rb == RB - 1))
                nc.scalar.activation(KT_sb[:, db, t, :], psk[:], AF.Copy)
            for sj in range(RB):
                sb = t * RB + sj
                ssl2 = slice(sj * P, (sj + 1) * P)
                psv = psSC.tile([P, DHG], F32, tag="sc")
                for rb in range(RB):
                    nc.tensor.matmul(psv[:], tg[:, rb, 2, ssl2], vw[:, rb, :],
                                     start=(rb == 0), stop=(rb == RB - 1))
                nc.scalar.activation(V_sb[:, sb, :], psv[:], AF.Copy)

        # ---- prologue ------------------------------------------------
        # Chunk-0 inputs + weights; full read matrices staged into the
        # not-yet-needed V_sb (qk) and wo_sb (v) regions for replicated A(0).
        xt0 = pool_x.tile([P, DB, SC], BF, tag="xt")
        for q4 in range(4):
            qsl = slice(q4 * 4, (q4 + 1) * 4)
            nc.scalar.dma_start(xt0[:, qsl, :], _r(io['xT'])[:, qsl, :SC])
            nc.sync.dma_start(V_sb[:, qsl, :], _r(io['qk_readTf'])[:, qsl, :])
            nc.sync.dma_start(wo_sb[:, qsl, :RANK],
                              _r(io['v_readTf'])[:, qsl, :])
        nc.sync.dma_start(qr[:], _r(io['qk_readT']))
        nc.sync.dma_start(vr[:], _r(io['v_readT']))
        nc.scalar.dma_start(qw[:], _r(io['qk_w']))
        nc.scalar.dma_start(vw[:], _r(io['v_w']))
        nc.scalar.dma_start(gsel_sb[:], io['gsel'])

        # A(0)/A(1) replicated: full-rank gated t locally, no collective,
        # so nothing waits on the ~56us first-collective NRT barrier.
        def stage_a_repl(t, xt):
            tgl = pool_tg.tile([P, RB, 3, SC], BF, tag="tg")
            for rb in range(RB):
                rsl = slice(rb * P, (rb + 1) * P)
                gsl = slice(t * SC, (t + 1) * SC)
                g0q = pool_g.tile([P, SC], BF, tag="gq")
                nc.scalar.dma_start(g0q[:], _r(io['gq0'])[:, rb, gsl])
                g0k = pool_g.tile([P, SC], BF, tag="gk")
                nc.scalar.dma_start(g0k[:], _r(io['gk0'])[:, rb, gsl])
                g0v = pool_g.tile([P, SC], BF, tag="gv")
                nc.scalar.dma_start(g0v[:], _r(io['gv0'])[:, rb, gsl])
                ps = psSC.tile([P, SC], F32, tag="sc")
                for db in range(DB):
                    nc.tensor.matmul(ps[:], V_sb[:, db, rsl], xt[:, db, :],
                                     start=(db == 0), stop=(db == DB - 1))
                nc.vector.tensor_tensor(tgl[:, rb, 0, :], ps[:], g0q[:],
                                        ALU.mult)
                nc.vector.tensor_tensor(tgl[:, rb, 1, :], ps[:], g0k[:],
                                        ALU.mult)
                psv = psSC.tile([P, SC], F32, tag="sc")
                for db in range(DB):
                    nc.tensor.matmul(psv[:], wo_sb[:, db, rsl], xt[:, db, :],
                                     start=(db == 0), stop=(db == DB - 1))
                nc.vector.tensor_tensor(tgl[:, rb, 2, :], psv[:], g0v[:],
                                        ALU.mult)
            return tgl

        tg0 = stage_a_repl(0, xt0)
        ins1 = dma_chunk_inputs(1, gates=False)
        tg1 = stage_a_repl(1, ins1[0])
        ins2 = dma_chunk_inputs(2)
        ins3 = dma_chunk_inputs(3)
        # W_O loads: emitted after A(0/1)'s reads of the staged region (WAR);
        # behind all chunk inputs on the Act HWDGE queue so xt never waits.
        for wq in range(4):
            nc.scalar.dma_start(wo_sb[:, wq * 4:(wq + 1) * 4, :],
                                _r(io['wo_full'])[:, wq * 4:(wq + 1) * 4, :])
        QT0 = pool_qt.tile([P, HG, SC], BF, tag="qt")
        stage_b(0, tg0, QT0)
        tg_local = {1: tg1}

        QT = QT0
        aot_prev = None
        for t in range(NT):
            # ---- C(t): attention for queries in chunk t, all own heads
            ao = pool_ao.tile([P, HG, SC], BF, tag="ao")
            npair = 2 * (t + 1)
            nquad = npair // 2

            def head_tail(h, pv, rs, e2args):
                """Finish head h: last quad rowsum, fast fp32 1/Z on DVE,
                f32r broadcast matmul, normalize."""
                e2, st, sp = e2args
                nc.tensor.matmul(rs[:], ones_r[:], e2[:], start=st, stop=sp)
                recip = pool_sm.tile([1, SC], F32, tag="recip")
                nc.vector.reciprocal_approx_fast(out=recip[:], in_=rs[:])
                recipb = pool_sm.tile([1, SC], BF, tag="recipb")
                nc.vector.tensor_copy(recipb[:], recip[:])
                rep = psSC.tile([P, SC], F32, tag="sc")
                nc.tensor.matmul(rep[:], onecol[:], recipb[:],
                                 start=True, stop=True)
                nc.scalar.activation(ao[:, h, :], pv[:], AF.Copy)
                nc.vector.tensor_tensor(ao[:, h, :], ao[:, h, :], rep[:],
                                        ALU.mult)

            aoz0 = pool_ao.tile([P, HG, SC], BF, tag="aoz0")
            aoz1 = pool_ao.tile([P, HG, SC], BF, tag="aoz1")

            def emit_a2a_half(half):
                """Exchange ao heads {2*half, 2*half+1} across the group."""
                hs = slice(2 * half, 2 * half + 2)
                nc.vector.tensor_scalar(aoz0[:, hs, :], ao[:, hs, :],
                                        gsel_sb[:, 0:1], None, ALU.mult)
                nc.vector.tensor_scalar(aoz1[:, hs, :], ao[:, hs, :],
                                        gsel_sb[:, 1:2], None, ALU.mult)
                for g2 in range(HG):
                    qsl = slice(g2 * P, (g2 + 1) * P)
                    nc.sync.dma_start(a2a_in[half][t, g2], aoz0[:, hs, qsl])
                    nc.sync.dma_start(a2a_in[half][t, HG + g2],
                                      aoz1[:, hs, qsl])
                nc.gpsimd.collective_compute(
                    "AllToAll", ALU.bypass, ins=[a2a_in[half][t].opt()],
                    outs=[a2a_out[half][t].opt()],
                    replica_groups=[list(range(8))])

            prev_tail = None
            for h in range(HG):
                if h == 3:
                    emit_a2a_half(0)   # heads 0,1 final; hide under C tail
                hsl = slice(h * P, (h + 1) * P)
                pv = psPV.tile([P, SC], F32, tag="pv")
                rs = psRS.tile([1, SC], F32, tag="rs")
                pend_rs = []    # one-pair-lagged quad rowsum matmuls
                ets_hold = None

                def sc_pair(q):
                    """Emit scores+exp(+mask) for pair q; return et tiles."""
                    etps = []
                    for k in range(2):
                        jb = 2 * q + k
                        jc, jp = divmod(jb, RB)
                        sc = psSC.tile([P, SC], F32, tag="sc")
                        nc.tensor.matmul(
                            sc[:], KT_sb[:, h, jc, jp * P:(jp + 1) * P],
                            QT[:, h, :], start=True, stop=True)
                        etp = pool_et.tile([P, SC], BF, tag="et")
                        nc.scalar.activation(etp[:], sc[:], AF.Exp,
                                             scale=EXP_SCALE)
                        o = jb - 4 * t
                        if o >= 0:
                            msl = slice(3 * P - P * o, 3 * P - P * o + SC)
                            nc.vector.tensor_tensor(etp[:], etp[:],
                                                    masks[:, msl], ALU.mult)
                        etps.append(etp)
                    return etps

                # scores run one pair ahead of PV so the exp latency is
                # hidden behind the previous pair's PV matmuls
                etp_cur = sc_pair(0)
                for q in range(npair):
                    etp_next = sc_pair(q + 1) if q + 1 < npair else None
                    if pend_rs:
                        e2, st, sp = pend_rs.pop()
                        nc.tensor.matmul(rs[:], ones_r[:], e2[:],
                                         start=st, stop=sp)
                    for k in range(2):
                        jb = 2 * q + k
                        nc.tensor.matmul(pv[:], V_sb[:, jb, hsl],
                                         etp_cur[k][:],
                                         start=(q == 0 and k == 0),
                                         stop=(q == npair - 1 and k == 1))
                    if prev_tail is not None:
                        head_tail(*prev_tail)   # overlap prior head's tail
                        prev_tail = None
                    ets = pool_ets.tile([P, SC], BF, tag="ets")
                    nc.vector.tensor_tensor(ets[:], etp_cur[0][:],
                                            etp_cur[1][:], ALU.add)
                    if q % 2 == 0:
                        ets_hold = ets
                    else:
                        qd = q // 2
                        ets2 = pool_ets.tile([P, SC], BF, tag="ets2")
                        nc.vector.tensor_tensor(ets2[:], ets_hold[:], ets[:],
                                                ALU.add)
                        pend_rs.append((ets2, qd == 0, qd == nquad - 1))
                    etp_cur = etp_next
                prev_tail = (h, pv, rs, pend_rs.pop())
            head_tail(*prev_tail)   # last head: must finish before ao DMA
            prev_tail = None

            # ---- A2A(t) second half (heads 2,3)
            emit_a2a_half(1)

            # ---- A(2) in the first A2A window: its AG trigger queues on
            # the CC stream BEHIND A2A(0), which D(0) needs much earlier
            # than B(2) needs this gather
            if t == 0:
                stage_a(2, ins2)

            # prefetch next chunk's gathered t before the blocking t1/t2
            # fetches take the SP queue
            tg_next = None
            if t + 1 < NT:
                tg_next = tg_local.get(t + 1) or tg_fetch(t + 1)

            # ---- receive A2A(t): fetch + add (the DVE adds block only
            # until the exchange lands; D(t-1) and B(t+1) need no DVE)
            t1 = pool_aot.tile([P, HG, HG, P], BF, tag="t1")
            t2 = pool_ao2.tile([P, HG, HG, P], BF, tag="t2")
            for half in range(2):
                hs = slice(2 * half, 2 * half + 2)
                nc.sync.dma_start(
                    t1[:, :, hs, :],
                    a2a_out[half][t, :HG].rearrange("g p h q -> p g h q"))
                nc.sync.dma_start(
                    t2[:, :, hs, :],
                    a2a_out[half][t, HG:].rearrange("g p h q -> p g h q"))
                nc.vector.tensor_tensor(t1[:, :, hs, :], t1[:, :, hs, :],
                                        t2[:, :, hs, :], ALU.add)

            # ---- D(t-1): deferred a full chunk so A2A(t-1) is long done
            if aot_prev is not None:
                out_sb = pool_osb.tile([P, HG, SC], BF, tag="osb")
                cbs = [g * HG + h for hp in ((0, 1), (2, 3))
                       for h in hp for g in range(HG)]
                for oc in range(4):
                    psd = psSC.tile([P, SC], F32, tag="sc")
                    for ci, cb in enumerate(cbs):
                        nc.tensor.matmul(
                            psd[:], aot_prev[:, cb // HG, cb % HG, :],
                            wo_sb[:, cb, oc * SC:(oc + 1) * SC],
                            start=(ci == 0), stop=(ci == DB - 1))
                    nc.vector.tensor_copy(out_sb[:, oc, :], psd[:])
                    nc.scalar.dma_start(
                        io['out'][t - 1][:, oc * SC:(oc + 1) * SC],
                        out_sb[:, oc, :])

            # ---- B(t+1)
            if t + 1 < NT:
                QT = pool_qt.tile([P, HG, SC], BF, tag="qt")
                stage_b(t + 1, tg_next, QT)
            if t == 0:
                stage_a(3, ins3)   # xt(3) DMA has landed by now
            aot_prev = t1

        # ---- D(3) + final out
        out_sb = pool_osb.tile([P, HG, SC], BF, tag="osb")
        cbs = [g * HG + h for hp in ((0, 1), (2, 3))
               for h in hp for g in range(HG)]
        for oc in range(4):
            psd = psSC.tile([P, SC], F32, tag="sc")
            for ci, cb in enumerate(cbs):
                nc.tensor.matmul(
                    psd[:], aot_prev[:, cb // HG, cb % HG, :],
                    wo_sb[:, cb, oc * SC:(oc + 1) * SC],
                    start=(ci == 0), stop=(ci == DB - 1))
            nc.vector.tensor_copy(out_sb[:, oc, :], psd[:])
            nc.scalar.dma_start(io['out'][NT - 1][:, oc * SC:(oc + 1) * SC],
                                out_sb[:, oc, :])


def _get_nc():
    if 'nc' not in _CACHE:
        _CACHE['nc'] = _build()
    return _CACHE['nc']


def _bf(a):
    return np.ascontiguousarray(np.asarray(a, np.float32)).astype(
        ml_dtypes.bfloat16)


def kernel(**inputs):
    x = np.asarray(inputs["x"], np.float32)
    g_Q = np.asarray(inputs["g_Q"], np.float32)
    g_K = np.asarray(inputs["g_K"], np.float32)
    g_V = np.asarray(inputs["g_V"], np.float32)
    qk_read = np.asarray(inputs["qk_read"], np.float32)
    qk_write = np.asarray(inputs["qk_write"], np.float32)
    v_read = np.asarray(inputs["v_read"], np.float32)
    v_write = np.asarray(inputs["v_write"], np.float32)
    W_O = np.asarray(inputs["W_O"], np.float32)

    nc = _get_nc()
    wo_b = _bf(W_O * INV_KEEP2)
    qk_readTf = _bf(qk_read.T)
    v_readTf = _bf(v_read.T)
    xTb = [_bf(x[b].T) for b in range(B)]
    gqTb = [_bf(g_Q[b].T) for b in range(B)]
    gkTb = [_bf(g_K[b].T) for b in range(B)]
    gvTb = [_bf(g_V[b].T) for b in range(B)]
    in_maps = []
    for c in range(8):
        b, g = divmod(c, 4)
        gsel_c = np.zeros((P, 2), np.float32)
        gsel_c[:, 0 if b == 0 else 1] = 1.0
        ssl = slice(g * DHG, (g + 1) * DHG)
        rsl = slice(g * P, (g + 1) * P)
        in_maps.append({
            "gsel": gsel_c,
            "xT": xTb[b],
            "gqT": np.ascontiguousarray(gqTb[b][rsl]),
            "gkT": np.ascontiguousarray(gkTb[b][rsl]),
            "gvT": np.ascontiguousarray(gvTb[b][rsl]),
            "gq0": np.ascontiguousarray(gqTb[b][:, :2 * SC]),
            "gk0": np.ascontiguousarray(gkTb[b][:, :2 * SC]),
            "gv0": np.ascontiguousarray(gvTb[b][:, :2 * SC]),
            "qk_readT": np.ascontiguousarray(qk_readTf[:, rsl]),
            "v_readT": np.ascontiguousarray(v_readTf[:, rsl]),
            "qk_readTf": qk_readTf,
            "v_readTf": v_readTf,
            "qk_w": _bf(qk_write[:, ssl]),
            "v_w": _bf(v_write[:, ssl]),
            "wo_full": wo_b,
        })
    res = run_bass_kernel_spmd(nc, in_maps, core_ids=list(range(8)))
    _CACHE['last_results'] = res
    out = np.empty((B, S, D), np.float32)
    for c in range(8):
        b, g = divmod(c, 4)
        o = np.asarray(res.results[c]["out"], dtype=ml_dtypes.bfloat16)
        for t in range(NT):
            r0 = t * SC + g * P
            out[b, r0:r0 + P, :] = o[t].astype(np.float32)
    return out


# revision 38
# speedup vs baseline: 1.0464x; 1.0464x over previous
"""Distributed Bass kernel for nn_AttentionCircuit (B=2,S=2048,D=2048,RANK=512,H=16).

Sharding: 8 cores = 2 batches x 4 group-positions. Core (b, g) computes
attention for head-group g (4 heads / 512 D-cols) of batch b over all S.

Two collectives per chunk, both off the critical path:
  - The gated low-rank projection t (A-stage) is rank-sharded: each core
    computes its own 128 rank rows (1/4 of the work) and an AllGather
    rebuilds the full [rank, SC] gated t (chunks 2/3; their AG triggers
    are emitted in iter 0 AFTER the first A2A so they queue behind it on
    the trigger-ordered CC stream). Chunks 0/1 are instead computed
    REPLICATED (full rank locally, read matrices staged in the
    not-yet-loaded V_sb/wo_sb regions) so B(0)/B(1)/C(0) never wait on
    the ~30-70us first-collective NRT barrier.
  - Instead of a ReduceScatter of [SC, D] W_O partials (the last one an
    unoverlapped ~64us tail), each chunk's attention output ao is
    AllToAll'd (over all 8 cores -- 4-core A2A is unsupported -- with
    each peer-slice dual-written to both candidate rank slots and the
    receiver keeping its own group's half via a per-core 0/1 gsel input)
    and every core computes its own 128 output rows against a full SBUF
    copy of W_O. Each A2A is split in two head-pair halves (the first
    fires mid-C(t) and is fully hidden), D(t) is deferred one chunk so
    the exchange is long done when it runs, and B(t+1) fills the A2A(t)
    window. Both collective paths (A2A at real payload size, AG) are
    warmed during the barrier shadow -- the first large A2A otherwise
    pays a ~30us ramp.
All matmul operands bf16 (1 cyc/row on PE, half the DMA bytes of fp32).
Bulk loads (x, gates, weights, W_O) go through the Activation HWDGE queue
so latency-critical collective DMAs on the SP queue never sit behind them.

Per-core, per 512-wide s-chunk: A (own rank rows of gated t^T) -> AG;
B: Q^T/K^T[own cols]/V from gathered t; C: scores^T = K^T.T Q^T -> exp ->
causal mask (block skip + static masks on diagonal) -> rowsum via
ones-matmul of DVE quad-sums (lagged one pair) -> PV -> normalize via
fast fp32 1/Z + bcast matmul (1/0.81 folded into W_O on host);
A2A: ao -> aoT_full [dh, 16 global heads, 128 own queries];
D: own out rows = aoT_full.T @ W_O (full 2048 contraction).
Host reassembles: core (b, g) holds rows t*512+g*128..+128 of batch b.
"""
import sys
import numpy as np
import ml_dtypes

sys.path.insert(0, '/opt/trn_rl_repo')

import concourse.bass as bass  # noqa: E402
from concourse import bacc  # noqa: E402
import concourse.mybir as mybir  # noqa: E402
import concourse.tile as tile  # noqa: E402
from concourse.bass_utils import run_bass_kernel_spmd  # noqa: E402

B, S, D = 2, 2048, 2048
RANK = 512
NH = 16
HG = 4              # heads per core / group size
DHG = D // HG       # 512 cols per core
P = 128
DB = D // P         # 16 d-blocks
RB = RANK // P      # 4 rank-blocks (== own-col blocks)
SC = S // 4         # 512: s-chunk width == t-chunk width
NT = S // SC        # 4 chunks

F32 = mybir.dt.float32
BF = mybir.dt.bfloat16
AF = mybir.ActivationFunctionType
ALU = mybir.AluOpType

EXP_SCALE = 1.0 / float(np.sqrt(P))
INV_KEEP2 = 1.0 / (0.9 * 0.9)
RGROUPS = [[0, 1, 2, 3], [4, 5, 6, 7]]

_CACHE = {}


def _r(ap):
    """[ (o p), f ] DRAM tensor -> [p, o, f] partition-tiled view."""
    return ap.rearrange("(o p) f -> p o f", p=P)


def _build():
    nc = bacc.Bacc("TRN2", target_bir_lowering=False, debug=False,
                   enable_asserts=False, num_devices=8)
    dt_ = nc.dram_tensor
    xT = dt_("xT", [D, S], BF, kind="ExternalInput").ap()
    gqT = dt_("gqT", [P, S], BF, kind="ExternalInput").ap()
    gkT = dt_("gkT", [P, S], BF, kind="ExternalInput").ap()
    gvT = dt_("gvT", [P, S], BF, kind="ExternalInput").ap()
    qk_readT = dt_("qk_readT", [D, P], BF, kind="ExternalInput").ap()
    v_readT = dt_("v_readT", [D, P], BF, kind="ExternalInput").ap()
    qk_readTf = dt_("qk_readTf", [D, RANK], BF, kind="ExternalInput").ap()
    v_readTf = dt_("v_readTf", [D, RANK], BF, kind="ExternalInput").ap()
    gq0 = dt_("gq0", [RANK, 2 * SC], BF, kind="ExternalInput").ap()
    gk0 = dt_("gk0", [RANK, 2 * SC], BF, kind="ExternalInput").ap()
    gv0 = dt_("gv0", [RANK, 2 * SC], BF, kind="ExternalInput").ap()
    qk_w = dt_("qk_w", [RANK, DHG], BF, kind="ExternalInput").ap()
    v_w = dt_("v_w", [RANK, DHG], BF, kind="ExternalInput").ap()
    wo_full = dt_("wo_full", [D, D], BF, kind="ExternalInput").ap()
    gsel = dt_("gsel", [P, 2], F32, kind="ExternalInput").ap()
    out = dt_("out", [NT, P, D], BF, kind="ExternalOutput").ap()

    with tile.TileContext(nc) as tc:
        _body(tc, dict(
            xT=xT, gqT=gqT, gkT=gkT, gvT=gvT, qk_readT=qk_readT,
            v_readT=v_readT, qk_readTf=qk_readTf, v_readTf=v_readTf,
            gq0=gq0, gk0=gk0, gv0=gv0, qk_w=qk_w, v_w=v_w,
            wo_full=wo_full, gsel=gsel, out=out))
    nc.compile()
    return nc


def _body(tc, io):
    nc = tc.nc
    import contextlib
    ctx = contextlib.ExitStack()
    with ctx:
        pool_main = ctx.enter_context(tc.tile_pool(name="main", bufs=1))
        pool_x = ctx.enter_context(tc.tile_pool(name="x", bufs=1))
        pool_g = ctx.enter_context(tc.tile_pool(name="g", bufs=1))
        pool_agin = ctx.enter_context(tc.tile_pool(name="agin", bufs=1))
        pool_tg = ctx.enter_context(tc.tile_pool(name="tg", bufs=2))
        pool_qt = ctx.enter_context(tc.tile_pool(name="qt", bufs=1))
        pool_ao = ctx.enter_context(tc.tile_pool(name="ao", bufs=1))
        pool_aot = ctx.enter_context(tc.tile_pool(name="aot", bufs=2))
        pool_ao2 = ctx.enter_context(tc.tile_pool(name="ao2", bufs=1))
        pool_osb = ctx.enter_context(tc.tile_pool(name="osb", bufs=1))
        pool_et = ctx.enter_context(tc.tile_pool(name="et", bufs=6))
        pool_ets = ctx.enter_context(tc.tile_pool(name="ets", bufs=2))
        pool_sm = ctx.enter_context(tc.tile_pool(name="sm", bufs=1))
        pool_dram = ctx.enter_context(tc.tile_pool(name="dramb", bufs=1,
                                                   space="DRAM"))
        psSC = ctx.enter_context(tc.tile_pool(name="psSC", bufs=5, space="PSUM"))
        psPV = ctx.enter_context(tc.tile_pool(name="psPV", bufs=2, space="PSUM"))
        psRS = ctx.enter_context(tc.tile_pool(name="psRS", bufs=1, space="PSUM"))

        # warm up the collective stream first: absorbs the ~45us NRT
        # first-collective barrier while the prologue computes chunk 0
        warm_in = pool_dram.tile([P, 1024], BF)
        warm_out = pool_dram.tile([RB, P, 1024], BF)
        warm2_in = pool_dram.tile([8, P, 2, P], BF)
        warm2_out = pool_dram.tile([8, P, 2, P], BF)
        nc.gpsimd.collective_compute(
            "AllToAll", ALU.bypass, ins=[warm2_in.opt()],
            outs=[warm2_out.opt()], replica_groups=[list(range(8))])
        nc.gpsimd.collective_compute(
            "AllGather", ALU.bypass, ins=[warm_in.opt()],
            outs=[warm_out.opt()], replica_groups=RGROUPS)

        # ---- long-lived tensors / constants
        KT_sb = pool_main.tile([P, HG, NT, SC], BF)   # K^T [dh, head, chunk, s]
        V_sb = pool_main.tile([P, DB, DHG], BF)       # V [s-block, own cols]
        wo_sb = pool_main.tile([P, DB, D], BF)        # full W_O (row-tiled)
        qr = pool_main.tile([P, DB, P], BF)           # own 128 rank cols
        vr = pool_main.tile([P, DB, P], BF)
        qw = pool_main.tile([P, RB, DHG], BF)
        vw = pool_main.tile([P, RB, DHG], BF)
        masks = pool_main.tile([P, SC + 3 * P], BF)   # sliding causal mask
        ones_r = pool_main.tile([P, 1], BF)
        onecol = pool_main.tile([1, P], BF)
        gsel_sb = pool_main.tile([P, 2], F32)

        nc.vector.memset(masks[:], 1.0)
        nc.gpsimd.affine_select(
            out=masks[:], in_=masks[:],
            compare_op=ALU.is_ge, fill=0.0, base=-3 * P,
            pattern=[[1, SC + 3 * P]], channel_multiplier=-1)
        nc.vector.memset(ones_r[:], 1.0)
        nc.vector.memset(onecol[:], 1.0)

        ag_in_d = pool_dram.tile([NT, P, 3, SC], BF)
        ag_out_d = pool_dram.tile([NT, RB, P, 3, SC], BF)
        a2a_in0 = pool_dram.tile([NT, 8, P, 2, P], BF, tag="a2a_in0")
        a2a_in1 = pool_dram.tile([NT, 8, P, 2, P], BF, tag="a2a_in1")
        a2a_out0 = pool_dram.tile([NT, 8, P, 2, P], BF, tag="a2a_out0")
        a2a_out1 = pool_dram.tile([NT, 8, P, 2, P], BF, tag="a2a_out1")
        a2a_in = [a2a_in0, a2a_in1]
        a2a_out = [a2a_out0, a2a_out1]

        def dma_chunk_inputs(t, gates=True):
            # chunks >=2 ride the SP queue, which idles after the prologue
            # staging; keeps the Act HWDGE queue free for chunks 0/1 + W_O
            eng = nc.sync if t >= 2 else nc.scalar
            csl = slice(t * SC, (t + 1) * SC)
            xt = pool_x.tile([P, DB, SC], BF, tag="xt")
            eng.dma_start(xt[:, :DB // 2, :], _r(io['xT'])[:, :DB // 2, csl])
            eng.dma_start(xt[:, DB // 2:, :], _r(io['xT'])[:, DB // 2:, csl])
            if not gates:
                return xt, None, None, None
            gq = pool_g.tile([P, SC], BF, tag="gq")
            eng.dma_start(gq[:], io['gqT'][:, csl])
            gk = pool_g.tile([P, SC], BF, tag="gk")
            eng.dma_start(gk[:], io['gkT'][:, csl])
            gv = pool_g.tile([P, SC], BF, tag="gv")
            eng.dma_start(gv[:], io['gvT'][:, csl])
            return xt, gq, gk, gv

        def stage_a(t, ins):
            """Own 128 rank rows of gated t^T for s-chunk t -> AllGather."""
            xt, gq, gk, gv = ins
            agi = pool_agin.tile([P, 3, SC], BF, tag="agi")
            ps = psSC.tile([P, SC], F32, tag="sc")
            for db in range(DB):
                nc.tensor.matmul(ps[:], qr[:, db, :], xt[:, db, :],
                                 start=(db == 0), stop=(db == DB - 1))
            nc.vector.tensor_tensor(agi[:, 0, :], ps[:], gq[:], ALU.mult)
            nc.vector.tensor_tensor(agi[:, 1, :], ps[:], gk[:], ALU.mult)
            psv = psSC.tile([P, SC], F32, tag="sc")
            for db in range(DB):
                nc.tensor.matmul(psv[:], vr[:, db, :], xt[:, db, :],
                                 start=(db == 0), stop=(db == DB - 1))
            nc.vector.tensor_tensor(agi[:, 2, :], psv[:], gv[:], ALU.mult)
            nc.sync.dma_start(ag_in_d[t], agi[:])
            nc.gpsimd.collective_compute(
                "AllGather", ALU.bypass, ins=[ag_in_d[t].opt()],
                outs=[ag_out_d[t].opt()], replica_groups=RGROUPS)

        def tg_fetch(t):
            tg = pool_tg.tile([P, RB, 3, SC], BF, tag="tg")
            nc.sync.dma_start(
                tg[:], ag_out_d[t].rearrange("g p c f -> p g c f"))
            return tg

        def stage_b(t, tg, QT_sb):
            """Q^T/K^T [own cols, chunk t], V [chunk t, own cols]."""
            for db in range(RB):
                dsl = slice(db * P, (db + 1) * P)
                psq = psSC.tile([P, SC], F32, tag="sc")
                for rb in range(RB):
                    nc.tensor.matmul(psq[:], qw[:, rb, dsl], tg[:, rb, 0, :],
                                     start=(rb == 0), stop=(rb == RB - 1))
                nc.scalar.activation(QT_sb[:, db, :], psq[:], AF.Copy)
                psk = psSC.tile([P, SC], F32, tag="sc")
                for rb in range(RB):
                    nc.tensor.matmul(psk[:], qw[:, rb, dsl], tg[:, rb, 1, :],
                                     start=(rb == 0), stop=(rb == RB - 1))
                nc.scalar.activation(KT_sb[:, db, t, :], psk[:], AF.Copy)
            for sj in range(RB):
                sb = t * RB + sj
                ssl2 = slice(sj * P, (sj + 1) * P)
                psv = psSC.tile([P, DHG], F32, tag="sc")
                for rb in range(RB):
                    nc.tensor.matmul(psv[:], tg[:, rb, 2, ssl2], vw[:, rb, :],
                                     start=(rb == 0), stop=(rb == RB - 1))
                nc.scalar.activation(V_sb[:, sb, :], psv[:], AF.Copy)

        # ---- prologue ------------------------------------------------
        # Chunk-0 inputs + weights; full read matrices staged into the
        # not-yet-needed V_sb (qk) and wo_sb (v) regions for replicated A(0).
        xt0 = pool_x.tile([P, DB, SC], BF, tag="xt")
        for q4 in range(4):
            qsl = slice(q4 * 4, (q4 + 1) * 4)
            nc.scalar.dma_start(xt0[:, qsl, :], _r(io['xT'])[:, qsl, :SC])
            nc.sync.dma_start(V_sb[:, qsl, :], _r(io['qk_readTf'])[:, qsl, :])
            nc.sync.dma_start(wo_sb[:, qsl, :RANK],
                              _r(io['v_readTf'])[:, qsl, :])
        nc.sync.dma_start(qr[:], _r(io['qk_readT']))
        nc.sync.dma_start(vr[:], _r(io['v_readT']))
        nc.sync.dma_start(qw[:], _r(io['qk_w']))
        nc.sync.dma_start(vw[:], _r(io['v_w']))
        nc.scalar.dma_start(gsel_sb[:], io['gsel'])

        # A(0)/A(1) replicated: full-rank gated t locally, no collective,
        # so nothing waits on the ~56us first-collective NRT barrier.
        def stage_a_repl(t, xt):
            tgl = pool_tg.tile([P, RB, 3, SC], BF, tag="tg")
            for rb in range(RB):
                rsl = slice(rb * P, (rb + 1) * P)
                gsl = slice(t * SC, (t + 1) * SC)
                g0q = pool_g.tile([P, SC], BF, tag="gq")
                nc.scalar.dma_start(g0q[:], _r(io['gq0'])[:, rb, gsl])
                g0k = pool_g.tile([P, SC], BF, tag="gk")
                nc.scalar.dma_start(g0k[:], _r(io['gk0'])[:, rb, gsl])
                g0v = pool_g.tile([P, SC], BF, tag="gv")
                nc.scalar.dma_start(g0v[:], _r(io['gv0'])[:, rb, gsl])
                ps = psSC.tile([P, SC], F32, tag="sc")
                for db in range(DB):
                    nc.tensor.matmul(ps[:], V_sb[:, db, rsl], xt[:, db, :],
                                     start=(db == 0), stop=(db == DB - 1))
                nc.vector.tensor_tensor(tgl[:, rb, 0, :], ps[:], g0q[:],
                                        ALU.mult)
                nc.vector.tensor_tensor(tgl[:, rb, 1, :], ps[:], g0k[:],
                                        ALU.mult)
                psv = psSC.tile([P, SC], F32, tag="sc")
                for db in range(DB):
                    nc.tensor.matmul(psv[:], wo_sb[:, db, rsl], xt[:, db, :],
                                     start=(db == 0), stop=(db == DB - 1))
                nc.vector.tensor_tensor(tgl[:, rb, 2, :], psv[:], g0v[:],
                                        ALU.mult)
            return tgl

        tg0 = stage_a_repl(0, xt0)
        ins1 = dma_chunk_inputs(1, gates=False)
        tg1 = stage_a_repl(1, ins1[0])
        ins2 = dma_chunk_inputs(2)
        ins3 = dma_chunk_inputs(3)
        # W_O loads: emitted after A(0/1)'s reads of the staged region (WAR);
        # behind all chunk inputs on the Act HWDGE queue so xt never waits.
        for wq in range(4):
            nc.scalar.dma_start(wo_sb[:, wq * 4:(wq + 1) * 4, :],
                                _r(io['wo_full'])[:, wq * 4:(wq + 1) * 4, :])
        QT0 = pool_qt.tile([P, HG, SC], BF, tag="qt")
        stage_b(0, tg0, QT0)
        tg_local = {1: tg1}

        QT = QT0
        aot_prev = None
        for t in range(NT):
            # ---- C(t): attention for queries in chunk t, all own heads
            ao = pool_ao.tile([P, HG, SC], BF, tag="ao")
            npair = 2 * (t + 1)
            nquad = npair // 2

            def head_tail(h, pv, rs, e2args):
                """Finish head h: last quad rowsum, fast fp32 1/Z on DVE,
                f32r broadcast matmul, normalize."""
                e2, st, sp = e2args
                nc.tensor.matmul(rs[:], ones_r[:], e2[:], start=st, stop=sp)
                recip = pool_sm.tile([1, SC], F32, tag="recip")
                nc.vector.reciprocal_approx_fast(out=recip[:], in_=rs[:])
                recipb = pool_sm.tile([1, SC], BF, tag="recipb")
                nc.vector.tensor_copy(recipb[:], recip[:])
                rep = psSC.tile([P, SC], F32, tag="sc")
                nc.tensor.matmul(rep[:], onecol[:], recipb[:],
                                 start=True, stop=True)
                nc.scalar.activation(ao[:, h, :], pv[:], AF.Copy)
                nc.vector.tensor_tensor(ao[:, h, :], ao[:, h, :], rep[:],
                                        ALU.mult)

            aoz0 = pool_ao.tile([P, HG, SC], BF, tag="aoz0")
            aoz1 = pool_ao.tile([P, HG, SC], BF, tag="aoz1")

            def emit_a2a_half(half):
                """Exchange ao heads {2*half, 2*half+1} across the group."""
                hs = slice(2 * half, 2 * half + 2)
                nc.vector.tensor_scalar(aoz0[:, hs, :], ao[:, hs, :],
                                        gsel_sb[:, 0:1], None, ALU.mult)
                nc.vector.tensor_scalar(aoz1[:, hs, :], ao[:, hs, :],
                                        gsel_sb[:, 1:2], None, ALU.mult)
                for g2 in range(HG):
                    qsl = slice(g2 * P, (g2 + 1) * P)
                    nc.sync.dma_start(a2a_in[half][t, g2], aoz0[:, hs, qsl])
                    nc.sync.dma_start(a2a_in[half][t, HG + g2],
                                      aoz1[:, hs, qsl])
                nc.gpsimd.collective_compute(
                    "AllToAll", ALU.bypass, ins=[a2a_in[half][t].opt()],
                    outs=[a2a_out[half][t].opt()],
                    replica_groups=[list(range(8))])

            prev_tail = None
            for h in range(HG):
                if h == 3:
                    emit_a2a_half(0)   # heads 0,1 final; hide under C tail
                hsl = slice(h * P, (h + 1) * P)
                pv = psPV.tile([P, SC], F32, tag="pv")
                rs = psRS.tile([1, SC], F32, tag="rs")
                pend_rs = []    # one-pair-lagged quad rowsum matmuls
                ets_hold = None

                def sc_pair(q):
                    """Emit scores+exp(+mask) for pair q; return et tiles."""
                    etps = []
                    for k in range(2):
                        jb = 2 * q + k
                        jc, jp = divmod(jb, RB)
                        sc = psSC.tile([P, SC], F32, tag="sc")
                        nc.tensor.matmul(
                            sc[:], KT_sb[:, h, jc, jp * P:(jp + 1) * P],
                            QT[:, h, :], start=True, stop=True)
                        etp = pool_et.tile([P, SC], BF, tag="et")
                        nc.scalar.activation(etp[:], sc[:], AF.Exp,
                                             scale=EXP_SCALE)
                        o = jb - 4 * t
                        if o >= 0:
                            msl = slice(3 * P - P * o, 3 * P - P * o + SC)
                            nc.vector.tensor_tensor(etp[:], etp[:],
                                                    masks[:, msl], ALU.mult)
                        etps.append(etp)
                    return etps

                # scores run one pair ahead of PV so the exp latency is
                # hidden behind the previous pair's PV matmuls
                etp_cur = sc_pair(0)
                for q in range(npair):
                    etp_next = sc_pair(q + 1) if q + 1 < npair else None
                    if pend_rs:
                        e2, st, sp = pend_rs.pop()
                        nc.tensor.matmul(rs[:], ones_r[:], e2[:],
                                         start=st, stop=sp)
                    for k in range(2):
                        jb = 2 * q + k
                        nc.tensor.matmul(pv[:], V_sb[:, jb, hsl],
                                         etp_cur[k][:],
                                         start=(q == 0 and k == 0),
                                         stop=(q == npair - 1 and k == 1))
                    if prev_tail is not None:
                        head_tail(*prev_tail)   # overlap prior head's tail
                        prev_tail = None
                    ets = pool_ets.tile([P, SC], BF, tag="ets")
                    nc.vector.tensor_tensor(ets[:], etp_cur[0][:],
                                            etp_cur[1][:], ALU.add)
                    if q % 2 == 0:
                        ets_hold = ets
                    else:
                        qd = q // 2
                        ets2 = pool_ets.tile([P, SC], BF, tag="ets2")
                        nc.vector.tensor_tensor(ets2[:], ets_hold[:], ets[:],
                                                ALU.add)
                        pend_rs.append((ets2, qd == 0, qd == nquad - 1))
                    etp_cur = etp_next
                prev_tail = (h, pv, rs, pend_rs.pop())
            head_tail(*prev_tail)   # last head: must finish before ao DMA
            prev_tail = None

            # ---- A2A(t) second half (heads 2,3)
            emit_a2a_half(1)

            # ---- A(2) in the first A2A window: its AG trigger queues on
            # the CC stream BEHIND A2A(0), which D(0) needs much earlier
            # than B(2) needs this gather
            if t == 0:
                stage_a(2, ins2)

            # prefetch next chunk's gathered t before the blocking t1/t2
            # fetches take the SP queue
            tg_next = None
            if t + 1 < NT:
                tg_next = tg_local.get(t + 1) or tg_fetch(t + 1)

            # ---- receive A2A(t): fetch + add (the DVE adds block only
            # until the exchange lands; D(t-1) and B(t+1) need no DVE)
            t1 = pool_aot.tile([P, HG, HG, P], BF, tag="t1")
            t2 = pool_ao2.tile([P, HG, HG, P], BF, tag="t2")
            for half in range(2):
                hs = slice(2 * half, 2 * half + 2)
                nc.sync.dma_start(
                    t1[:, :, hs, :],
                    a2a_out[half][t, :HG].rearrange("g p h q -> p g h q"))
                nc.sync.dma_start(
                    t2[:, :, hs, :],
                    a2a_out[half][t, HG:].rearrange("g p h q -> p g h q"))
                nc.vector.tensor_tensor(t1[:, :, hs, :], t1[:, :, hs, :],
                                        t2[:, :, hs, :], ALU.add)

            # ---- D(t-1): deferred a full chunk so A2A(t-1) is long done
            if aot_prev is not None:
                out_sb = pool_osb.tile([P, HG, SC], BF, tag="osb")
                cbs = [g * HG + h for hp in ((0, 1), (2, 3))
                       for h in hp for g in range(HG)]
                for oc in range(4):
                    psd = psSC.tile([P, SC], F32, tag="sc")
                    for ci, cb in enumerate(cbs):
                        nc.tensor.matmul(
                            psd[:], aot_prev[:, cb // HG, cb % HG, :],
                            wo_sb[:, cb, oc * SC:(oc + 1) * SC],
                            start=(ci == 0), stop=(ci == DB - 1))
                    nc.vector.tensor_copy(out_sb[:, oc, :], psd[:])
                    nc.scalar.dma_start(
                        io['out'][t - 1][:, oc * SC:(oc + 1) * SC],
                        out_sb[:, oc, :])

            # ---- B(t+1)
            if t + 1 < NT:
                QT = pool_qt.tile([P, HG, SC], BF, tag="qt")
                stage_b(t + 1, tg_next, QT)
            if t == 0:
                stage_a(3, ins3)   # xt(3) DMA has landed by now
            aot_prev = t1

        # ---- D(3) + final out
        out_sb = pool_osb.tile([P, HG, SC], BF, tag="osb")
        cbs = [g * HG + h for hp in ((0, 1), (2, 3))
               for h in hp for g in range(HG)]
        for oc in range(4):
            psd = psSC.tile([P, SC], F32, tag="sc")
            for ci, cb in enumerate(cbs):
                nc.tensor.matmul(
                    psd[:], aot_prev[:, cb // HG, cb % HG, :],
                    wo_sb[:, cb, oc * SC:(oc + 1) * SC],
                    start=(ci == 0), stop=(ci == DB - 1))
            nc.vector.tensor_copy(out_sb[:, oc, :], psd[:])
            nc.scalar.dma_start(io['out'][NT - 1][:, oc * SC:(oc + 1) * SC],
                                out_sb[:, oc, :])


def _get_nc():
    if 'nc' not in _CACHE:
        _CACHE['nc'] = _build()
    return _CACHE['nc']


def _bf(a):
    return np.ascontiguousarray(np.asarray(a, np.float32)).astype(
        ml_dtypes.bfloat16)


def kernel(**inputs):
    x = np.asarray(inputs["x"], np.float32)
    g_Q = np.asarray(inputs["g_Q"], np.float32)
    g_K = np.asarray(inputs["g_K"], np.float32)
    g_V = np.asarray(inputs["g_V"], np.float32)
    qk_read = np.asarray(inputs["qk_read"], np.float32)
    qk_write = np.asarray(inputs["qk_write"], np.float32)
    v_read = np.asarray(inputs["v_read"], np.float32)
    v_write = np.asarray(inputs["v_write"], np.float32)
    W_O = np.asarray(inputs["W_O"], np.float32)

    nc = _get_nc()
    wo_b = _bf(W_O * INV_KEEP2)
    qk_readTf = _bf(qk_read.T)
    v_readTf = _bf(v_read.T)
    xTb = [_bf(x[b].T) for b in range(B)]
    gqTb = [_bf(g_Q[b].T) for b in range(B)]
    gkTb = [_bf(g_K[b].T) for b in range(B)]
    gvTb = [_bf(g_V[b].T) for b in range(B)]
    in_maps = []
    for c in range(8):
        b, g = divmod(c, 4)
        gsel_c = np.zeros((P, 2), np.float32)
        gsel_c[:, 0 if b == 0 else 1] = 1.0
        ssl = slice(g * DHG, (g + 1) * DHG)
        rsl = slice(g * P, (g + 1) * P)
        in_maps.append({
            "gsel": gsel_c,
            "xT": xTb[b],
            "gqT": np.ascontiguousarray(gqTb[b][rsl]),
            "gkT": np.ascontiguousarray(gkTb[b][rsl]),
            "gvT": np.ascontiguousarray(gvTb[b][rsl]),
            "gq0": np.ascontiguousarray(gqTb[b][:, :2 * SC]),
            "gk0": np.ascontiguousarray(gkTb[b][:, :2 * SC]),
            "gv0": np.ascontiguousarray(gvTb[b][:, :2 * SC]),
            "qk_readT": np.ascontiguousarray(qk_readTf[:, rsl]),
            "v_readT": np.ascontiguousarray(v_readTf[:, rsl]),
            "qk_readTf": qk_readTf,
            "v_readTf": v_readTf,
            "qk_w": _bf(qk_write[:, ssl]),
            "v_w": _bf(v_write[:, ssl]),
            "wo_full": wo_b,
        })
    res = run_bass_kernel_spmd(nc, in_maps, core_ids=list(range(8)))
    _CACHE['last_results'] = res
    out = np.empty((B, S, D), np.float32)
    for c in range(8):
        b, g = divmod(c, 4)
        o = np.asarray(res.results[c]["out"], dtype=ml_dtypes.bfloat16)
        for t in range(NT):
            r0 = t * SC + g * P
            out[b, r0:r0 + P, :] = o[t].astype(np.float32)
    return out
